# revision 49
# baseline (speedup 1.0000x reference)
"""Cross-attention kernel for 8 TRN2 NeuronCores.

Strategy: pure data-parallel over batch B=64 -> 8 batches/core, all
activations feature-major ([features, tokens]).

Key structure (v3):
- RoPE is applied as q_rope = cos*y + sin*(R @ y) with the fixed pair
  rotation done by one PE matmul per tile (blockdiag(R^T,R^T) stationary);
  the rot matmul is deferred into the next chain's emission slot so the PE
  never waits on the ACT evacuation of y.
- attn@V runs feature-major: stationary [V | ones] (65 cols) with the
  exp'd logits as moving operand; psum row 64 is the softmax denominator,
  applied by reciprocal + partition_broadcast + one DVE multiply that
  writes the attention output directly feature-major (no transposes).
- The PE instruction stream is software-pipelined: group g's Q/K/V
  projection chains are interleaved with group g-1's attention matmuls so
  the tensor engine stays continuously busy (keeps the PE at its top
  p-state and hides softmax latency).
- Only input staging goes through the sync queue; output stores are issued
  by the Scalar engine right after producing each tile so no dependent DMA
  ever blocks the sync sequencer.

Compute dtype: bf16 operands, fp32 PSUM accumulation; softmax in fp32.
"""

import numpy as np
import ml_dtypes
from contextlib import ExitStack

import concourse.bass as bass
import concourse.tile as tile
from concourse import bacc, mybir
from concourse.bass_utils import run_bass_kernel_spmd

# ---- problem constants (hardcoded per contract) ----
B, N, C, SEM = 64, 256, 1024, 768
H, HD = 16, 64
NCORES = 8
BPC = B // NCORES          # batches per core
T = BPC * N                # tokens per core (2048)
P = 128
KQ = C // P                # 8 contraction tiles for q-proj
KS = SEM // P              # 6 contraction tiles for kv-proj
M = C // P                 # 8 output-feature tiles
G = 4                      # token groups per core
GT = T // G                # tokens per group (512)
NB = 2                     # batches per group
PT_SEQ_LEN = 16
THETA = 10000.0

BF = mybir.dt.bfloat16
F32 = mybir.dt.float32
bf16 = ml_dtypes.bfloat16


def _rope_tables_np():
    d = HD // 2                                         # 32
    freqs = 1.0 / (THETA ** (np.arange(0, d, 2, dtype=np.float64) / d))   # (16,)
    t = np.arange(PT_SEQ_LEN, dtype=np.float64)
    f = np.einsum('i,j->ij', t, freqs)                  # (16, 16)
    f = np.repeat(f, 2, axis=-1)                        # (16, 32)
    fa = np.broadcast_to(f[:, None, :], (PT_SEQ_LEN, PT_SEQ_LEN, d))
    fb = np.broadcast_to(f[None, :, :], (PT_SEQ_LEN, PT_SEQ_LEN, d))
    full = np.concatenate([fa, fb], axis=-1).reshape(-1, HD)   # (256, 64)
    return np.cos(full).astype(np.float32), np.sin(full).astype(np.float32)


def _host_constants():
    cos, sin = _rope_tables_np()                        # (256, 64) each
    cosT = np.ascontiguousarray(cos.T)                  # (64, 256)
    sinT = np.ascontiguousarray(sin.T)
    cosrep = np.tile(cosT, (2, 2))                      # (128, 512)
    sinrep = np.tile(sinT, (2, 2))
    scale = 1.0 / np.sqrt(np.float32(HD))               # folded into q side
    consts = {
        "cosq": (cosrep * scale).astype(bf16),
        "sinq": (sinrep * scale).astype(bf16),
        "cosk": cosrep.astype(bf16),
        "sink": sinrep.astype(bf16),
    }
    # RT2 = blockdiag(R^T, R^T): psum = RT2.T @ y = rot(y)
    RT = np.zeros((HD, HD), np.float32)
    for i in range(HD // 2):
        RT[2 * i + 1, 2 * i] = -1.0
        RT[2 * i, 2 * i + 1] = 1.0
    RT2 = np.zeros((P, P), np.float32)
    RT2[:HD, :HD] = RT
    RT2[HD:, HD:] = RT
    consts["RT2"] = RT2.astype(bf16)
    return consts


def _act_reciprocal(nc, out, in_):
    """ACT-engine reciprocal (bass blocks the wrapper for accuracy reasons;
    the softmax denominator here is a benign-range positive sum and the
    result only normalizes attention weights, so table accuracy suffices)."""
    inputs = [nc.scalar.lower_ap(in_)]
    for v in (0.0, 1.0, 0.0):  # bias, scale, alpha
        inputs.append(mybir.ImmediateValue(dtype=mybir.dt.float32, value=v))
    return nc.scalar.add_instruction(
        mybir.InstActivation(
            name=nc.scalar.bass.get_next_instruction_name(),
            func=mybir.ActivationFunctionType.Reciprocal,
            ins=inputs,
            outs=[nc.scalar.lower_ap(out)],
        ))


def _merge(a, b):
    """Proportional round-robin merge of two work lists."""
    out = []
    ia = ib = 0
    la, lb = len(a), len(b)
    while ia < la or ib < lb:
        if ib >= lb or (ia < la and ia * lb <= ib * la):
            out.append(a[ia]); ia += 1
        else:
            out.append(b[ib]); ib += 1
    return out


def _body(ctx: ExitStack, tc: "tile.TileContext", io: dict):
    nc = tc.nc

    wpool = ctx.enter_context(tc.tile_pool(name="weights", bufs=1))
    const = ctx.enter_context(tc.tile_pool(name="const", bufs=1))
    inq = ctx.enter_context(tc.tile_pool(name="inq", bufs=2))
    inkv = ctx.enter_context(tc.tile_pool(name="inkv", bufs=2))
    acts = ctx.enter_context(tc.tile_pool(name="acts", bufs=2))
    aop = ctx.enter_context(tc.tile_pool(name="aop", bufs=2))
    ytmp = ctx.enter_context(tc.tile_pool(name="ytmp", bufs=4))
    tmp = ctx.enter_context(tc.tile_pool(name="tmp", bufs=4))
    expp = ctx.enter_context(tc.tile_pool(name="expp", bufs=8))
    rvp = ctx.enter_context(tc.tile_pool(name="rvp", bufs=4))
    rvbp = ctx.enter_context(tc.tile_pool(name="rvbp", bufs=4))
    outp = ctx.enter_context(tc.tile_pool(name="outp", bufs=3))
    ps_proj = ctx.enter_context(tc.tile_pool(name="ps_proj", bufs=3, space="PSUM"))
    ps_log = ctx.enter_context(tc.tile_pool(name="ps_log", bufs=1, space="PSUM"))
    ps_av = ctx.enter_context(tc.tile_pool(name="ps_av", bufs=3, space="PSUM"))

    staged = {}
    pending_dma = {}

    def stage_group(g):
        """Allocate group-g input tiles; defer the dma_starts to D items so
        the HBM traffic spreads across the iteration instead of bursting."""
        if g in staged or g >= G:
            return []
        c0 = g * GT
        qTg, kvTg, dmas = [], [], []
        for k in range(KQ):
            t_ = inq.tile([P, GT], BF, tag=f"qTg{k}")
            dmas.append((t_, io["qT"][k * P:(k + 1) * P, c0:c0 + GT]))
            qTg.append(t_)
        for k in range(KS):
            t_ = inkv.tile([P, GT], BF, tag=f"kvTg{k}")
            dmas.append((t_, io["kvT"][k * P:(k + 1) * P, c0:c0 + GT]))
            kvTg.append(t_)
        staged[g] = (qTg, kvTg)
        pending_dma[g] = dmas
        return [("D", g, i) for i in range(len(dmas))]

    # group-0 staging interleaved with the weights that the first chains
    # consume, in first-use order: the first q chain needs Wq[k]+qT[k],
    # then k chains need Wk[k]+kvT[k]; Wv/Wp follow later in the schedule.
    Wq_sb, qTg0, kvTg0 = [], [], []
    Wk_sb, Wv_sb = [], []
    for k in range(KQ):
        t_ = wpool.tile([P, C], BF, tag=f"wq{k}")
        nc.sync.dma_start(t_[:], io["Wq"][k * P:(k + 1) * P, :])
        Wq_sb.append(t_)
        t_ = inq.tile([P, GT], BF, tag=f"qTg{k}")
        nc.sync.dma_start(t_[:], io["qT"][k * P:(k + 1) * P, 0:GT])
        qTg0.append(t_)
        if k < KS:
            t_ = wpool.tile([P, C], BF, tag=f"wk{k}")
            nc.sync.dma_start(t_[:], io["Wk"][k * P:(k + 1) * P, :])
            Wk_sb.append(t_)
            t_ = inkv.tile([P, GT], BF, tag=f"kvTg{k}")
            nc.sync.dma_start(t_[:], io["kvT"][k * P:(k + 1) * P, 0:GT])
            kvTg0.append(t_)
    cn = {}
    for name, shape in [("cosq", [P, GT]), ("sinq", [P, GT]),
                        ("cosk", [P, GT]), ("sink", [P, GT]),
                        ("RT2", [P, P])]:
        t_ = const.tile(shape, BF, tag=name)
        nc.sync.dma_start(t_[:], io[name][:])
        cn[name] = t_
    bprojT = const.tile([P, M], F32, tag="bprojT")
    nc.sync.dma_start(bprojT[:], io["bprojT"][:])
    staged[0] = (qTg0, kvTg0)
    for k in range(KS):
        t_ = wpool.tile([P, C], BF, tag=f"wv{k}")
        nc.sync.dma_start(t_[:], io["Wv"][k * P:(k + 1) * P, :])
        Wv_sb.append(t_)
    Wp_sb = []
    for k in range(M):
        t_ = wpool.tile([P, C], BF, tag=f"wp{k}")
        nc.sync.dma_start(t_[:], io["Wproj"][k * P:(k + 1) * P, :])
        Wp_sb.append(t_)

    state = {}
    pending_rot = []

    def flush_rot():
        while pending_rot:
            pending_rot.pop(0)()

    def make_group(g):
        qrope = acts.tile([P, M, GT], BF, tag="qrope")
        krope = acts.tile([P, M, GT], BF, tag="krope")
        Vt = acts.tile([P, 4, H, HD + 1], BF, tag="Vt")
        nc.vector.memset(Vt[:, :, :, HD:HD + 1], 1.0)
        ao = aop.tile([P, M, GT], BF, tag="ao")
        state[g] = {"qrope": qrope, "krope": krope, "Vt": Vt, "ao": ao,
                    "exp": {}}

    def proj_chain(g, kind, idx):
        st = state[g]
        qTg, kvTg = staged[g]
        if kind == "v":
            tt, nn = divmod(idx, 2)
            acc = ps_proj.tile([P, GT], F32, tag="acc")
            for k in range(KS):
                nc.tensor.matmul(
                    acc[:], kvTg[k][:, tt * P:(tt + 1) * P],
                    Wv_sb[k][:, nn * GT:(nn + 1) * GT],
                    start=(k == 0), stop=(k == KS - 1))
            flush_rot()
            nc.scalar.copy(
                st["Vt"][:, tt, nn * 8:(nn + 1) * 8, 0:HD],
                acc[:].rearrange("p (h d) -> p h d", d=HD))
            return
        m = idx
        if kind == "q":
            dst, W, src, nk = st["qrope"], Wq_sb, qTg, KQ
            cosA, sinA = cn["cosq"], cn["sinq"]
        else:
            dst, W, src, nk = st["krope"], Wk_sb, kvTg, KS
            cosA, sinA = cn["cosk"], cn["sink"]
        acc = ps_proj.tile([P, GT], F32, tag="acc")
        for k in range(nk):
            nc.tensor.matmul(acc[:], W[k][:, m * P:(m + 1) * P], src[k][:],
                             start=(k == 0), stop=(k == nk - 1))
        flush_rot()
        y = ytmp.tile([P, GT], BF, tag="y")
        nc.scalar.copy(y[:], acc[:])
        t1 = tmp.tile([P, GT], BF, tag="t1")
        nc.vector.tensor_mul(t1[:], y[:], cosA[:])

        def rot():
            # rot reuses the accumulation bank (start=True overwrites);
            # t2 reads the rot result straight from PSUM on DVE
            nc.tensor.matmul(acc[:], cn["RT2"][:], y[:], start=True, stop=True)
            t2 = tmp.tile([P, GT], BF, tag="t2", name="t2")
            nc.vector.tensor_mul(t2[:], acc[:], sinA[:])
            nc.vector.tensor_add(dst[:, m, :], t1[:], t2[:])
        pending_rot.append(rot)

    def logits_step(g, bb, hp):
        st = state[g]
        t0 = bb * N
        qrope, krope = st["qrope"], st["krope"]
        # one 2-bank tile: sub (head parity) is bank-aligned so the
        # row-tiled sub0/sub1 matmul pairs still hit different banks
        psl = ps_log.tile([P, 2, 2, N], F32, tag="psl")
        for kt in range(2):
            for sub in range(2):
                p0 = sub * HD
                nc.tensor.matmul(
                    psl[:, sub, kt, :],
                    krope[p0:p0 + HD, hp, t0 + kt * P: t0 + (kt + 1) * P],
                    qrope[p0:p0 + HD, hp, t0:t0 + N],
                    start=True, stop=True)
        expT = expp.tile([P, 2, 2, N], BF, tag="expT")
        nc.scalar.activation(expT[:], psl[:],
                             mybir.ActivationFunctionType.Exp)
        st["exp"][(bb, hp)] = expT

    def av_step(g, bb, hp):
        st = state[g]
        t0 = bb * N
        Vt, ao = st["Vt"], st["ao"]
        expT = st["exp"].pop((bb, hp))
        avp = ps_av.tile([P, 2, N], F32, tag="avp")
        for sub in range(2):
            h = 2 * hp + sub
            for kt in range(2):
                nc.tensor.matmul(
                    avp[0:HD + 1, sub, :], Vt[:, bb * 2 + kt, h, :],
                    expT[:, sub, kt, :], start=(kt == 0), stop=(kt == 1))
        rv = rvp.tile([1, 2, N], F32, tag="rv")
        den = rvp.tile([1, 2, N], F32, tag="den", name="den")
        rvb = rvbp.tile([HD, 2, N], F32, tag="rvb")
        nc.scalar.copy(den[0:1, :, :], avp[HD:HD + 1, :, :])
        nc.vector.reciprocal_approx_fast(out=rv[0:1, :, :],
                                         in_=den[0:1, :, :])
        nc.gpsimd.partition_broadcast(rvb[0:HD, :, :], rv[0:1, :, :])
        for sub in range(2):
            nc.vector.tensor_mul(
                ao[sub * HD:(sub + 1) * HD, hp, t0:t0 + N],
                avp[0:HD, sub, :], rvb[0:HD, sub, :])

    def oproj_chain(g, m):
        st = state[g]
        ao = st["ao"]
        psf = ps_proj.tile([P, GT], F32, tag="acc")
        for k2 in range(M):
            nc.tensor.matmul(psf[:], Wp_sb[k2][:, m * P:(m + 1) * P],
                             ao[:, k2, :], start=(k2 == 0), stop=(k2 == M - 1))
        osb = outp.tile([P, GT], BF, tag="osb")
        nc.scalar.add(osb[:], psf[:], add=bprojT[:, m:m + 1])
        nc.sync.dma_start(
            io["outT"][m * P:(m + 1) * P, g * GT:(g + 1) * GT], osb[:])

    def attn_items(g):
        items = []
        for bb in range(NB):
            for hp in range(M):
                items.append(("L", g, bb, hp))
                if hp >= 1:
                    items.append(("A", g, bb, hp - 1))
            items.append(("A", g, bb, M - 1))
        return items

    def oproj_items(g):
        return [("O", g, m) for m in range(M)]

    def proj_items(g):
        items = []
        for m in range(M):
            items.append(("Pq", g, m))
            items.append(("Pk", g, m))
        for i in range(M):
            items.append(("Pv", g, i))
        return items

    def emit(item):
        kind, g, *rest = item
        if kind == "Pq":
            proj_chain(g, "q", rest[0])
        elif kind == "Pk":
            proj_chain(g, "k", rest[0])
        elif kind == "Pv":
            proj_chain(g, "v", rest[0])
        elif kind == "L":
            logits_step(g, rest[0], rest[1])
        elif kind == "A":
            av_step(g, rest[0], rest[1])
        elif kind == "O":
            oproj_chain(g, rest[0])
        elif kind == "D":
            t_, src = pending_dma[g][rest[0]]
            nc.sync.dma_start(t_[:], src)

    # ---- software-pipelined emission: proj(g) | attn(g-1) | oproj(g-2) ----
    for it in range(G + 2):
        g_proj = it if it < G else None
        g_attn = it - 1 if 1 <= it <= G else None
        g_out = it - 2 if it >= 2 else None
        if g_proj is not None:
            items_d = stage_group(g_proj + 1)
            make_group(g_proj)
            items_p = _merge(proj_items(g_proj), items_d)
        else:
            items_p = []
        items_o = oproj_items(g_out) if g_out is not None else []
        items_a = attn_items(g_attn) if g_attn is not None else []
        for item in _merge(items_a, _merge(items_p, items_o)):
            emit(item)
        flush_rot()
        if g_proj is not None:
            staged.pop(g_proj, None)
            pending_dma.pop(g_proj, None)
        if g_out is not None:
            state.pop(g_out, None)


_CACHED_NC = None


def _build_nc():
    global _CACHED_NC
    if _CACHED_NC is not None:
        return _CACHED_NC
    nc = bacc.Bacc("TRN2", target_bir_lowering=False, debug=False,
                   num_devices=NCORES)
    io = {}
    def din(name, shape, dt=BF):
        io[name] = nc.dram_tensor(name, shape, dt, kind="ExternalInput").ap()
    din("qT", [C, T])
    din("kvT", [SEM, T])
    din("Wq", [C, C])
    din("Wk", [SEM, C])
    din("Wv", [SEM, C])
    din("Wproj", [C, C])
    din("cosq", [P, GT]); din("sinq", [P, GT])
    din("cosk", [P, GT]); din("sink", [P, GT])
    din("RT2", [P, P])
    din("bprojT", [P, M], F32)
    io["outT"] = nc.dram_tensor("outT", [C, T], BF, kind="ExternalOutput").ap()

    with tile.TileContext(nc) as tc:
        with ExitStack() as ctx:
            _body(ctx, tc, io)
    nc.compile()
    _CACHED_NC = nc
    return nc


def kernel(q, kv, Wq, Wkv, Wproj, bproj, _trace=False, _trace_kwargs=None):
    nc = _build_nc()
    consts = _host_constants()
    shared = {
        "Wq": np.ascontiguousarray(Wq.astype(bf16)),
        "Wk": np.ascontiguousarray(Wkv[:, :C].astype(bf16)),
        "Wv": np.ascontiguousarray(Wkv[:, C:].astype(bf16)),
        "Wproj": np.ascontiguousarray(Wproj.astype(bf16)),
        "bprojT": np.ascontiguousarray(
            bproj.astype(np.float32).reshape(M, P).T),
        **consts,
    }
    in_maps = []
    for i in range(NCORES):
        qs = q[i * BPC:(i + 1) * BPC].reshape(T, C)
        kvs = kv[i * BPC:(i + 1) * BPC].reshape(T, SEM)
        in_maps.append({
            "qT": np.ascontiguousarray(qs.T.astype(bf16)),
            "kvT": np.ascontiguousarray(kvs.T.astype(bf16)),
            **shared,
        })
    kw = {}
    if _trace:
        kw.update(trace=True, **(_trace_kwargs or {}))
    res = run_bass_kernel_spmd(nc, in_maps, core_ids=list(range(NCORES)), **kw)
    out = np.empty((B, N, C), np.float32)
    for i in range(NCORES):
        out[i * BPC:(i + 1) * BPC] = (
            res.results[i]["outT"].astype(np.float32).T.reshape(BPC, N, C))
    if _trace:
        return out, res
    return out



# revision 50
# speedup vs baseline: 1.0010x; 1.0010x over previous
"""Cross-attention kernel for 8 TRN2 NeuronCores.

Strategy: pure data-parallel over batch B=64 -> 8 batches/core, all
activations feature-major ([features, tokens]).

Key structure (v3):
- RoPE is applied as q_rope = cos*y + sin*(R @ y) with the fixed pair
  rotation done by one PE matmul per tile (blockdiag(R^T,R^T) stationary);
  the rot matmul is deferred into the next chain's emission slot so the PE
  never waits on the ACT evacuation of y.
- attn@V runs feature-major: stationary [V | ones] (65 cols) with the
  exp'd logits as moving operand; psum row 64 is the softmax denominator,
  applied by reciprocal + partition_broadcast + one DVE multiply that
  writes the attention output directly feature-major (no transposes).
- The PE instruction stream is software-pipelined: group g's Q/K/V
  projection chains are interleaved with group g-1's attention matmuls so
  the tensor engine stays continuously busy (keeps the PE at its top
  p-state and hides softmax latency).
- Only input staging goes through the sync queue; output stores are issued
  by the Scalar engine right after producing each tile so no dependent DMA
  ever blocks the sync sequencer.

Compute dtype: bf16 operands, fp32 PSUM accumulation; softmax in fp32.
"""

import numpy as np
import ml_dtypes
from contextlib import ExitStack

import concourse.bass as bass
import concourse.tile as tile
from concourse import bacc, mybir
from concourse.bass_utils import run_bass_kernel_spmd

# ---- problem constants (hardcoded per contract) ----
B, N, C, SEM = 64, 256, 1024, 768
H, HD = 16, 64
NCORES = 8
BPC = B // NCORES          # batches per core
T = BPC * N                # tokens per core (2048)
P = 128
KQ = C // P                # 8 contraction tiles for q-proj
KS = SEM // P              # 6 contraction tiles for kv-proj
M = C // P                 # 8 output-feature tiles
G = 4                      # token groups per core
GT = T // G                # tokens per group (512)
NB = 2                     # batches per group
PT_SEQ_LEN = 16
THETA = 10000.0

BF = mybir.dt.bfloat16
F32 = mybir.dt.float32
bf16 = ml_dtypes.bfloat16


def _rope_tables_np():
    d = HD // 2                                         # 32
    freqs = 1.0 / (THETA ** (np.arange(0, d, 2, dtype=np.float64) / d))   # (16,)
    t = np.arange(PT_SEQ_LEN, dtype=np.float64)
    f = np.einsum('i,j->ij', t, freqs)                  # (16, 16)
    f = np.repeat(f, 2, axis=-1)                        # (16, 32)
    fa = np.broadcast_to(f[:, None, :], (PT_SEQ_LEN, PT_SEQ_LEN, d))
    fb = np.broadcast_to(f[None, :, :], (PT_SEQ_LEN, PT_SEQ_LEN, d))
    full = np.concatenate([fa, fb], axis=-1).reshape(-1, HD)   # (256, 64)
    return np.cos(full).astype(np.float32), np.sin(full).astype(np.float32)


def _host_constants():
    cos, sin = _rope_tables_np()                        # (256, 64) each
    cosT = np.ascontiguousarray(cos.T)                  # (64, 256)
    sinT = np.ascontiguousarray(sin.T)
    cosrep = np.tile(cosT, (2, 2))                      # (128, 512)
    sinrep = np.tile(sinT, (2, 2))
    scale = 1.0 / np.sqrt(np.float32(HD))               # folded into q side
    consts = {
        "cosq": (cosrep * scale).astype(bf16),
        "sinq": (sinrep * scale).astype(bf16),
        "cosk": cosrep.astype(bf16),
        "sink": sinrep.astype(bf16),
    }
    # RT2 = blockdiag(R^T, R^T): psum = RT2.T @ y = rot(y)
    RT = np.zeros((HD, HD), np.float32)
    for i in range(HD // 2):
        RT[2 * i + 1, 2 * i] = -1.0
        RT[2 * i, 2 * i + 1] = 1.0
    RT2 = np.zeros((P, P), np.float32)
    RT2[:HD, :HD] = RT
    RT2[HD:, HD:] = RT
    consts["RT2"] = RT2.astype(bf16)
    return consts


def _act_reciprocal(nc, out, in_):
    """ACT-engine reciprocal (bass blocks the wrapper for accuracy reasons;
    the softmax denominator here is a benign-range positive sum and the
    result only normalizes attention weights, so table accuracy suffices)."""
    inputs = [nc.scalar.lower_ap(in_)]
    for v in (0.0, 1.0, 0.0):  # bias, scale, alpha
        inputs.append(mybir.ImmediateValue(dtype=mybir.dt.float32, value=v))
    return nc.scalar.add_instruction(
        mybir.InstActivation(
            name=nc.scalar.bass.get_next_instruction_name(),
            func=mybir.ActivationFunctionType.Reciprocal,
            ins=inputs,
            outs=[nc.scalar.lower_ap(out)],
        ))


def _merge(a, b):
    """Proportional round-robin merge of two work lists."""
    out = []
    ia = ib = 0
    la, lb = len(a), len(b)
    while ia < la or ib < lb:
        if ib >= lb or (ia < la and ia * lb <= ib * la):
            out.append(a[ia]); ia += 1
        else:
            out.append(b[ib]); ib += 1
    return out


def _body(ctx: ExitStack, tc: "tile.TileContext", io: dict):
    nc = tc.nc

    wpool = ctx.enter_context(tc.tile_pool(name="weights", bufs=1))
    const = ctx.enter_context(tc.tile_pool(name="const", bufs=1))
    inq = ctx.enter_context(tc.tile_pool(name="inq", bufs=2))
    inkv = ctx.enter_context(tc.tile_pool(name="inkv", bufs=2))
    acts = ctx.enter_context(tc.tile_pool(name="acts", bufs=2))
    aop = ctx.enter_context(tc.tile_pool(name="aop", bufs=2))
    ytmp = ctx.enter_context(tc.tile_pool(name="ytmp", bufs=4))
    tmp = ctx.enter_context(tc.tile_pool(name="tmp", bufs=4))
    expp = ctx.enter_context(tc.tile_pool(name="expp", bufs=8))
    rvp = ctx.enter_context(tc.tile_pool(name="rvp", bufs=4))
    rvbp = ctx.enter_context(tc.tile_pool(name="rvbp", bufs=4))
    outp = ctx.enter_context(tc.tile_pool(name="outp", bufs=3))
    ps_proj = ctx.enter_context(tc.tile_pool(name="ps_proj", bufs=3, space="PSUM"))
    ps_log = ctx.enter_context(tc.tile_pool(name="ps_log", bufs=1, space="PSUM"))
    ps_av = ctx.enter_context(tc.tile_pool(name="ps_av", bufs=3, space="PSUM"))

    staged = {}

    def stage_group(g):
        if g in staged or g >= G:
            return
        c0 = g * GT
        qTg = []
        for k in range(KQ):
            t_ = inq.tile([P, GT], BF, tag=f"qTg{k}")
            nc.sync.dma_start(t_[:], io["qT"][k * P:(k + 1) * P, c0:c0 + GT])
            qTg.append(t_)
        kvTg = []
        for k in range(KS):
            t_ = inkv.tile([P, GT], BF, tag=f"kvTg{k}")
            nc.sync.dma_start(t_[:], io["kvT"][k * P:(k + 1) * P, c0:c0 + GT])
            kvTg.append(t_)
        staged[g] = (qTg, kvTg)

    # group-0 staging interleaved with the weights that the first chains
    # consume, in first-use order: the first q chain needs Wq[k]+qT[k],
    # then k chains need Wk[k]+kvT[k]; Wv/Wp follow later in the schedule.
    Wq_sb, qTg0, kvTg0 = [], [], []
    Wk_sb, Wv_sb = [], []
    for k in range(KQ):
        t_ = wpool.tile([P, C], BF, tag=f"wq{k}")
        nc.sync.dma_start(t_[:], io["Wq"][k * P:(k + 1) * P, :])
        Wq_sb.append(t_)
        t_ = inq.tile([P, GT], BF, tag=f"qTg{k}")
        nc.sync.dma_start(t_[:], io["qT"][k * P:(k + 1) * P, 0:GT])
        qTg0.append(t_)
        if k < KS:
            t_ = wpool.tile([P, C], BF, tag=f"wk{k}")
            nc.sync.dma_start(t_[:], io["Wk"][k * P:(k + 1) * P, :])
            Wk_sb.append(t_)
            t_ = inkv.tile([P, GT], BF, tag=f"kvTg{k}")
            nc.sync.dma_start(t_[:], io["kvT"][k * P:(k + 1) * P, 0:GT])
            kvTg0.append(t_)
    cn = {}
    for name, shape in [("cosq", [P, GT]), ("sinq", [P, GT]),
                        ("cosk", [P, GT]), ("sink", [P, GT]),
                        ("RT2", [P, P])]:
        t_ = const.tile(shape, BF, tag=name)
        nc.sync.dma_start(t_[:], io[name][:])
        cn[name] = t_
    bprojT = const.tile([P, M], F32, tag="bprojT")
    nc.sync.dma_start(bprojT[:], io["bprojT"][:])
    staged[0] = (qTg0, kvTg0)
    for k in range(KS):
        t_ = wpool.tile([P, C], BF, tag=f"wv{k}")
        nc.sync.dma_start(t_[:], io["Wv"][k * P:(k + 1) * P, :])
        Wv_sb.append(t_)
    Wp_sb = []
    for k in range(M):
        t_ = wpool.tile([P, C], BF, tag=f"wp{k}")
        nc.sync.dma_start(t_[:], io["Wproj"][k * P:(k + 1) * P, :])
        Wp_sb.append(t_)

    state = {}
    pending_rot = []

    def flush_rot():
        while pending_rot:
            pending_rot.pop(0)()

    def make_group(g):
        qrope = acts.tile([P, M, GT], BF, tag="qrope")
        krope = acts.tile([P, M, GT], BF, tag="krope")
        Vt = acts.tile([P, 4, H, HD + 1], BF, tag="Vt")
        nc.vector.memset(Vt[:, :, :, HD:HD + 1], 1.0)
        ao = aop.tile([P, M, GT], BF, tag="ao")
        state[g] = {"qrope": qrope, "krope": krope, "Vt": Vt, "ao": ao,
                    "exp": {}}

    def proj_chain(g, kind, idx):
        st = state[g]
        qTg, kvTg = staged[g]
        if kind == "v":
            tt, nn = divmod(idx, 2)
            acc = ps_proj.tile([P, GT], F32, tag="acc")
            for k in range(KS):
                nc.tensor.matmul(
                    acc[:], kvTg[k][:, tt * P:(tt + 1) * P],
                    Wv_sb[k][:, nn * GT:(nn + 1) * GT],
                    start=(k == 0), stop=(k == KS - 1))
            flush_rot()
            nc.scalar.copy(
                st["Vt"][:, tt, nn * 8:(nn + 1) * 8, 0:HD],
                acc[:].rearrange("p (h d) -> p h d", d=HD))
            return
        m = idx
        if kind == "q":
            dst, W, src, nk = st["qrope"], Wq_sb, qTg, KQ
            cosA, sinA = cn["cosq"], cn["sinq"]
        else:
            dst, W, src, nk = st["krope"], Wk_sb, kvTg, KS
            cosA, sinA = cn["cosk"], cn["sink"]
        acc = ps_proj.tile([P, GT], F32, tag="acc")
        for k in range(nk):
            nc.tensor.matmul(acc[:], W[k][:, m * P:(m + 1) * P], src[k][:],
                             start=(k == 0), stop=(k == nk - 1))
        flush_rot()
        y = ytmp.tile([P, GT], BF, tag="y")
        nc.scalar.copy(y[:], acc[:])
        t1 = tmp.tile([P, GT], BF, tag="t1")
        nc.vector.tensor_mul(t1[:], y[:], cosA[:])

        def rot():
            # rot reuses the accumulation bank (start=True overwrites);
            # t2 reads the rot result straight from PSUM on DVE
            nc.tensor.matmul(acc[:], cn["RT2"][:], y[:], start=True, stop=True)
            t2 = tmp.tile([P, GT], BF, tag="t2", name="t2")
            nc.vector.tensor_mul(t2[:], acc[:], sinA[:])
            nc.vector.tensor_add(dst[:, m, :], t1[:], t2[:])
        pending_rot.append(rot)

    def logits_step(g, bb, hp):
        st = state[g]
        t0 = bb * N
        qrope, krope = st["qrope"], st["krope"]
        # one 2-bank tile: sub (head parity) is bank-aligned so the
        # row-tiled sub0/sub1 matmul pairs still hit different banks
        psl = ps_log.tile([P, 2, 2, N], F32, tag="psl")
        for kt in range(2):
            for sub in range(2):
                p0 = sub * HD
                nc.tensor.matmul(
                    psl[:, sub, kt, :],
                    krope[p0:p0 + HD, hp, t0 + kt * P: t0 + (kt + 1) * P],
                    qrope[p0:p0 + HD, hp, t0:t0 + N],
                    start=True, stop=True)
        expT = expp.tile([P, 2, 2, N], BF, tag="expT")
        nc.scalar.activation(expT[:], psl[:],
                             mybir.ActivationFunctionType.Exp)
        st["exp"][(bb, hp)] = expT

    def av_step(g, bb, hp):
        st = state[g]
        t0 = bb * N
        Vt, ao = st["Vt"], st["ao"]
        expT = st["exp"].pop((bb, hp))
        avp = ps_av.tile([P, 2, N], F32, tag="avp")
        for sub in range(2):
            h = 2 * hp + sub
            for kt in range(2):
                nc.tensor.matmul(
                    avp[0:HD + 1, sub, :], Vt[:, bb * 2 + kt, h, :],
                    expT[:, sub, kt, :], start=(kt == 0), stop=(kt == 1))
        rv = rvp.tile([1, 2, N], F32, tag="rv")
        den = rvp.tile([1, 2, N], F32, tag="den", name="den")
        rvb = rvbp.tile([HD, 2, N], F32, tag="rvb")
        nc.scalar.copy(den[0:1, :, :], avp[HD:HD + 1, :, :])
        nc.vector.reciprocal_approx_fast(out=rv[0:1, :, :],
                                         in_=den[0:1, :, :])
        nc.gpsimd.partition_broadcast(rvb[0:HD, :, :], rv[0:1, :, :])
        for sub in range(2):
            nc.vector.tensor_mul(
                ao[sub * HD:(sub + 1) * HD, hp, t0:t0 + N],
                avp[0:HD, sub, :], rvb[0:HD, sub, :])

    def oproj_chain(g, m):
        st = state[g]
        ao = st["ao"]
        psf = ps_proj.tile([P, GT], F32, tag="acc")
        for k2 in range(M):
            nc.tensor.matmul(psf[:], Wp_sb[k2][:, m * P:(m + 1) * P],
                             ao[:, k2, :], start=(k2 == 0), stop=(k2 == M - 1))
        osb = outp.tile([P, GT], BF, tag="osb")
        nc.scalar.add(osb[:], psf[:], add=bprojT[:, m:m + 1])
        nc.sync.dma_start(
            io["outT"][m * P:(m + 1) * P, g * GT:(g + 1) * GT], osb[:])

    def attn_items(g):
        items = []
        for bb in range(NB):
            for hp in range(M):
                items.append(("L", g, bb, hp))
                if hp >= 1:
                    items.append(("A", g, bb, hp - 1))
            items.append(("A", g, bb, M - 1))
        return items

    def oproj_items(g):
        return [("O", g, m) for m in range(M)]

    def proj_items(g):
        items = []
        for m in range(M):
            items.append(("Pq", g, m))
            items.append(("Pk", g, m))
        for i in range(M):
            items.append(("Pv", g, i))
        return items

    def emit(item):
        kind, g, *rest = item
        if kind == "Pq":
            proj_chain(g, "q", rest[0])
        elif kind == "Pk":
            proj_chain(g, "k", rest[0])
        elif kind == "Pv":
            proj_chain(g, "v", rest[0])
        elif kind == "L":
            logits_step(g, rest[0], rest[1])
        elif kind == "A":
            av_step(g, rest[0], rest[1])
        elif kind == "O":
            oproj_chain(g, rest[0])

    # ---- software-pipelined emission: proj(g) | attn(g-1) | oproj(g-2) ----
    for it in range(G + 2):
        g_proj = it if it < G else None
        g_attn = it - 1 if 1 <= it <= G else None
        g_out = it - 2 if it >= 2 else None
        if g_proj is not None:
            stage_group(g_proj + 1)
            make_group(g_proj)
            items_p = proj_items(g_proj)
        else:
            items_p = []
        items_o = oproj_items(g_out) if g_out is not None else []
        items_a = attn_items(g_attn) if g_attn is not None else []
        for item in _merge(items_a, _merge(items_p, items_o)):
            emit(item)
        flush_rot()
        if g_proj is not None:
            staged.pop(g_proj, None)
        if g_out is not None:
            state.pop(g_out, None)


_CACHED_NC = None


def _build_nc():
    global _CACHED_NC
    if _CACHED_NC is not None:
        return _CACHED_NC
    nc = bacc.Bacc("TRN2", target_bir_lowering=False, debug=False,
                   num_devices=NCORES)
    io = {}
    def din(name, shape, dt=BF):
        io[name] = nc.dram_tensor(name, shape, dt, kind="ExternalInput").ap()
    din("qT", [C, T])
    din("kvT", [SEM, T])
    din("Wq", [C, C])
    din("Wk", [SEM, C])
    din("Wv", [SEM, C])
    din("Wproj", [C, C])
    din("cosq", [P, GT]); din("sinq", [P, GT])
    din("cosk", [P, GT]); din("sink", [P, GT])
    din("RT2", [P, P])
    din("bprojT", [P, M], F32)
    io["outT"] = nc.dram_tensor("outT", [C, T], BF, kind="ExternalOutput").ap()

    with tile.TileContext(nc) as tc:
        with ExitStack() as ctx:
            _body(ctx, tc, io)
    nc.compile()
    _CACHED_NC = nc
    return nc


def kernel(q, kv, Wq, Wkv, Wproj, bproj, _trace=False, _trace_kwargs=None):
    nc = _build_nc()
    consts = _host_constants()
    shared = {
        "Wq": np.ascontiguousarray(Wq.astype(bf16)),
        "Wk": np.ascontiguousarray(Wkv[:, :C].astype(bf16)),
        "Wv": np.ascontiguousarray(Wkv[:, C:].astype(bf16)),
        "Wproj": np.ascontiguousarray(Wproj.astype(bf16)),
        "bprojT": np.ascontiguousarray(
            bproj.astype(np.float32).reshape(M, P).T),
        **consts,
    }
    in_maps = []
    for i in range(NCORES):
        qs = q[i * BPC:(i + 1) * BPC].reshape(T, C)
        kvs = kv[i * BPC:(i + 1) * BPC].reshape(T, SEM)
        in_maps.append({
            "qT": np.ascontiguousarray(qs.T.astype(bf16)),
            "kvT": np.ascontiguousarray(kvs.T.astype(bf16)),
            **shared,
        })
    kw = {}
    if _trace:
        kw.update(trace=True, **(_trace_kwargs or {}))
    res = run_bass_kernel_spmd(nc, in_maps, core_ids=list(range(NCORES)), **kw)
    out = np.empty((B, N, C), np.float32)
    for i in range(NCORES):
        out[i * BPC:(i + 1) * BPC] = (
            res.results[i]["outT"].astype(np.float32).T.reshape(BPC, N, C))
    if _trace:
        return out, res
    return out



# revision 51
# speedup vs baseline: 1.0315x; 1.0304x over previous
"""Cross-attention kernel for 8 TRN2 NeuronCores.

Strategy: pure data-parallel over batch B=64 -> 8 batches/core, all
activations feature-major ([features, tokens]).

Key structure (v3):
- RoPE is applied as q_rope = cos*y + sin*(R @ y) with the fixed pair
  rotation done by one PE matmul per tile (blockdiag(R^T,R^T) stationary);
  the rot matmul is deferred into the next chain's emission slot so the PE
  never waits on the ACT evacuation of y.
- attn@V runs feature-major: stationary [V | ones] (65 cols) with the
  exp'd logits as moving operand; psum row 64 is the softmax denominator,
  applied by reciprocal + partition_broadcast + one DVE multiply that
  writes the attention output directly feature-major (no transposes).
- The PE instruction stream is software-pipelined: group g's Q/K/V
  projection chains are interleaved with group g-1's attention matmuls so
  the tensor engine stays continuously busy (keeps the PE at its top
  p-state and hides softmax latency).
- Only input staging goes through the sync queue; output stores are issued
  by the Scalar engine right after producing each tile so no dependent DMA
  ever blocks the sync sequencer.

Compute dtype: bf16 operands, fp32 PSUM accumulation; softmax in fp32.
"""

import numpy as np
import ml_dtypes
from contextlib import ExitStack

import concourse.bass as bass
import concourse.tile as tile
from concourse import bacc, mybir
from concourse.bass_utils import run_bass_kernel_spmd

# ---- problem constants (hardcoded per contract) ----
B, N, C, SEM = 64, 256, 1024, 768
H, HD = 16, 64
NCORES = 8
BPC = B // NCORES          # batches per core
T = BPC * N                # tokens per core (2048)
P = 128
KQ = C // P                # 8 contraction tiles for q-proj
KS = SEM // P              # 6 contraction tiles for kv-proj
M = C // P                 # 8 output-feature tiles
G = 4                      # token groups per core
GT = T // G                # tokens per group (512)
NB = 2                     # batches per group
PT_SEQ_LEN = 16
THETA = 10000.0

BF = mybir.dt.bfloat16
F32 = mybir.dt.float32
bf16 = ml_dtypes.bfloat16


def _rope_tables_np():
    d = HD // 2                                         # 32
    freqs = 1.0 / (THETA ** (np.arange(0, d, 2, dtype=np.float64) / d))   # (16,)
    t = np.arange(PT_SEQ_LEN, dtype=np.float64)
    f = np.einsum('i,j->ij', t, freqs)                  # (16, 16)
    f = np.repeat(f, 2, axis=-1)                        # (16, 32)
    fa = np.broadcast_to(f[:, None, :], (PT_SEQ_LEN, PT_SEQ_LEN, d))
    fb = np.broadcast_to(f[None, :, :], (PT_SEQ_LEN, PT_SEQ_LEN, d))
    full = np.concatenate([fa, fb], axis=-1).reshape(-1, HD)   # (256, 64)
    return np.cos(full).astype(np.float32), np.sin(full).astype(np.float32)


def _host_constants():
    cos, sin = _rope_tables_np()                        # (256, 64) each
    cosT = np.ascontiguousarray(cos.T)                  # (64, 256)
    sinT = np.ascontiguousarray(sin.T)
    cosrep = np.tile(cosT, (2, 2))                      # (128, 512)
    sinrep = np.tile(sinT, (2, 2))
    scale = 1.0 / np.sqrt(np.float32(HD))               # folded into q side
    consts = {
        "cosq": (cosrep * scale).astype(bf16),
        "sinq": (sinrep * scale).astype(bf16),
        "cosk": cosrep.astype(bf16),
        "sink": sinrep.astype(bf16),
    }
    # RT2 = blockdiag(R^T, R^T): psum = RT2.T @ y = rot(y)
    RT = np.zeros((HD, HD), np.float32)
    for i in range(HD // 2):
        RT[2 * i + 1, 2 * i] = -1.0
        RT[2 * i, 2 * i + 1] = 1.0
    RT2 = np.zeros((P, P), np.float32)
    RT2[:HD, :HD] = RT
    RT2[HD:, HD:] = RT
    consts["RT2"] = RT2.astype(bf16)
    return consts


def _act_reciprocal(nc, out, in_):
    """ACT-engine reciprocal (bass blocks the wrapper for accuracy reasons;
    the softmax denominator here is a benign-range positive sum and the
    result only normalizes attention weights, so table accuracy suffices)."""
    inputs = [nc.scalar.lower_ap(in_)]
    for v in (0.0, 1.0, 0.0):  # bias, scale, alpha
        inputs.append(mybir.ImmediateValue(dtype=mybir.dt.float32, value=v))
    return nc.scalar.add_instruction(
        mybir.InstActivation(
            name=nc.scalar.bass.get_next_instruction_name(),
            func=mybir.ActivationFunctionType.Reciprocal,
            ins=inputs,
            outs=[nc.scalar.lower_ap(out)],
        ))


def _merge(a, b):
    """Proportional round-robin merge of two work lists."""
    out = []
    ia = ib = 0
    la, lb = len(a), len(b)
    while ia < la or ib < lb:
        if ib >= lb or (ia < la and ia * lb <= ib * la):
            out.append(a[ia]); ia += 1
        else:
            out.append(b[ib]); ib += 1
    return out


def _body(ctx: ExitStack, tc: "tile.TileContext", io: dict):
    nc = tc.nc

    wpool = ctx.enter_context(tc.tile_pool(name="weights", bufs=1))
    const = ctx.enter_context(tc.tile_pool(name="const", bufs=1))
    inq = ctx.enter_context(tc.tile_pool(name="inq", bufs=2))
    inkv = ctx.enter_context(tc.tile_pool(name="inkv", bufs=2))
    acts = ctx.enter_context(tc.tile_pool(name="acts", bufs=2))
    aop = ctx.enter_context(tc.tile_pool(name="aop", bufs=2))
    ytmp = ctx.enter_context(tc.tile_pool(name="ytmp", bufs=4))
    tmp = ctx.enter_context(tc.tile_pool(name="tmp", bufs=4))
    expp = ctx.enter_context(tc.tile_pool(name="expp", bufs=8))
    rvp = ctx.enter_context(tc.tile_pool(name="rvp", bufs=4))
    rvbp = ctx.enter_context(tc.tile_pool(name="rvbp", bufs=4))
    outp = ctx.enter_context(tc.tile_pool(name="outp", bufs=3))
    ps_proj = ctx.enter_context(tc.tile_pool(name="ps_proj", bufs=3, space="PSUM"))
    ps_log = ctx.enter_context(tc.tile_pool(name="ps_log", bufs=1, space="PSUM"))
    ps_av = ctx.enter_context(tc.tile_pool(name="ps_av", bufs=3, space="PSUM"))

    staged = {}

    def stage_group(g):
        if g in staged or g >= G:
            return
        c0 = g * GT
        qTg = []
        for k in range(KQ):
            t_ = inq.tile([P, GT], BF, tag=f"qTg{k}")
            nc.sync.dma_start(t_[:], io["qT"][k * P:(k + 1) * P, c0:c0 + GT])
            qTg.append(t_)
        kvTg = []
        for k in range(KS):
            t_ = inkv.tile([P, GT], BF, tag=f"kvTg{k}")
            nc.sync.dma_start(t_[:], io["kvT"][k * P:(k + 1) * P, c0:c0 + GT])
            kvTg.append(t_)
        staged[g] = (qTg, kvTg)

    # group-0 staging interleaved with the weights that the first chains
    # consume, in first-use order: the first q chain needs Wq[k]+qT[k],
    # then k chains need Wk[k]+kvT[k]; Wv/Wp follow later in the schedule.
    Wq_sb, qTg0, kvTg0 = [], [], []
    Wk_sb, Wv_sb = [], []
    for k in range(KQ):
        t_ = wpool.tile([P, C], BF, tag=f"wq{k}")
        nc.sync.dma_start(t_[:], io["Wq"][k * P:(k + 1) * P, :])
        Wq_sb.append(t_)
        t_ = inq.tile([P, GT], BF, tag=f"qTg{k}")
        nc.sync.dma_start(t_[:], io["qT"][k * P:(k + 1) * P, 0:GT])
        qTg0.append(t_)
        if k < KS:
            t_ = wpool.tile([P, C], BF, tag=f"wk{k}")
            nc.sync.dma_start(t_[:], io["Wk"][k * P:(k + 1) * P, :])
            Wk_sb.append(t_)
            t_ = inkv.tile([P, GT], BF, tag=f"kvTg{k}")
            nc.sync.dma_start(t_[:], io["kvT"][k * P:(k + 1) * P, 0:GT])
            kvTg0.append(t_)
    cn = {}
    for name, shape in [("cosq", [P, GT]), ("sinq", [P, GT]),
                        ("cosk", [P, GT]), ("sink", [P, GT]),
                        ("RT2", [P, P])]:
        t_ = const.tile(shape, BF, tag=name)
        nc.sync.dma_start(t_[:], io[name][:])
        cn[name] = t_
    bprojT = const.tile([P, M], F32, tag="bprojT")
    nc.sync.dma_start(bprojT[:], io["bprojT"][:])
    staged[0] = (qTg0, kvTg0)
    for k in range(KS):
        t_ = wpool.tile([P, C], BF, tag=f"wv{k}")
        nc.sync.dma_start(t_[:], io["Wv"][k * P:(k + 1) * P, :])
        Wv_sb.append(t_)
    Wp_sb = []
    for k in range(M):
        t_ = wpool.tile([P, C], BF, tag=f"wp{k}")
        nc.sync.dma_start(t_[:], io["Wproj"][k * P:(k + 1) * P, :])
        Wp_sb.append(t_)

    state = {}
    pending_rot = []

    def flush_rot():
        while pending_rot:
            pending_rot.pop(0)()

    def make_group(g):
        qrope = acts.tile([P, M, GT], BF, tag="qrope")
        krope = acts.tile([P, M, GT], BF, tag="krope")
        Vt = acts.tile([P, 4, H, HD + 1], BF, tag="Vt")
        nc.vector.memset(Vt[:, :, :, HD:HD + 1], 1.0)
        ao = aop.tile([P, M, GT], BF, tag="ao")
        state[g] = {"qrope": qrope, "krope": krope, "Vt": Vt, "ao": ao,
                    "exp": {}}

    def proj_chain(g, kind, idx):
        st = state[g]
        qTg, kvTg = staged[g]
        if kind == "v":
            tt, nn = divmod(idx, 2)
            acc = ps_proj.tile([P, GT], F32, tag="acc")
            for k in range(KS):
                nc.tensor.matmul(
                    acc[:], kvTg[k][:, tt * P:(tt + 1) * P],
                    Wv_sb[k][:, nn * GT:(nn + 1) * GT],
                    start=(k == 0), stop=(k == KS - 1))
            flush_rot()
            nc.scalar.copy(
                st["Vt"][:, tt, nn * 8:(nn + 1) * 8, 0:HD],
                acc[:].rearrange("p (h d) -> p h d", d=HD))
            return
        m = idx
        if kind == "q":
            dst, W, src, nk = st["qrope"], Wq_sb, qTg, KQ
            cosA, sinA = cn["cosq"], cn["sinq"]
        else:
            dst, W, src, nk = st["krope"], Wk_sb, kvTg, KS
            cosA, sinA = cn["cosk"], cn["sink"]
        acc = ps_proj.tile([P, GT], F32, tag="acc")
        for k in range(nk):
            nc.tensor.matmul(acc[:], W[k][:, m * P:(m + 1) * P], src[k][:],
                             start=(k == 0), stop=(k == nk - 1))
        flush_rot()
        y = ytmp.tile([P, GT], BF, tag="y")
        nc.scalar.copy(y[:], acc[:])
        t1 = tmp.tile([P, GT], BF, tag="t1")
        nc.vector.tensor_mul(t1[:], y[:], cosA[:])

        def rot():
            # rot reuses the accumulation bank (start=True overwrites);
            # t2 reads the rot result straight from PSUM on DVE
            nc.tensor.matmul(acc[:], cn["RT2"][:], y[:], start=True, stop=True)
            t2 = tmp.tile([P, GT], BF, tag="t2", name="t2")
            nc.vector.tensor_mul(t2[:], acc[:], sinA[:])
            nc.vector.tensor_add(dst[:, m, :], t1[:], t2[:])
        pending_rot.append(rot)

    def logits_step(g, bb, hp):
        st = state[g]
        t0 = bb * N
        qrope, krope = st["qrope"], st["krope"]
        # one 2-bank tile: sub (head parity) is bank-aligned so the
        # row-tiled sub0/sub1 matmul pairs still hit different banks
        psl = ps_log.tile([P, 2, 2, N], F32, tag="psl")
        for kt in range(2):
            for sub in range(2):
                p0 = sub * HD
                nc.tensor.matmul(
                    psl[:, sub, kt, :],
                    krope[p0:p0 + HD, hp, t0 + kt * P: t0 + (kt + 1) * P],
                    qrope[p0:p0 + HD, hp, t0:t0 + N],
                    start=True, stop=True)
        expT = expp.tile([P, 2, 2, N], BF, tag="expT")
        nc.scalar.activation(expT[:], psl[:],
                             mybir.ActivationFunctionType.Exp)
        st["exp"][(bb, hp)] = expT

    def av_step(g, bb, hp):
        st = state[g]
        t0 = bb * N
        Vt, ao = st["Vt"], st["ao"]
        expT = st["exp"].pop((bb, hp))
        avp = ps_av.tile([P, 2, N], F32, tag="avp")
        for sub in range(2):
            h = 2 * hp + sub
            for kt in range(2):
                nc.tensor.matmul(
                    avp[0:HD + 1, sub, :], Vt[:, bb * 2 + kt, h, :],
                    expT[:, sub, kt, :], start=(kt == 0), stop=(kt == 1))
        den = rvp.tile([1, 2, N], F32, tag="den", name="den")
        denb = rvbp.tile([HD, 2, N], F32, tag="denb", name="denb")
        rvb = rvbp.tile([HD, 2, N], F32, tag="rvb")
        nc.scalar.copy(den[0:1, :, :], avp[HD:HD + 1, :, :])
        # broadcast the raw denominator, then reciprocal on the broadcast:
        # recip and the normalize multiplies sit adjacent in the DVE queue,
        # so the per-step critical path has one fewer cross-engine hop
        nc.gpsimd.partition_broadcast(denb[0:HD, :, :], den[0:1, :, :])
        nc.vector.reciprocal_approx_fast(out=rvb[0:HD, :, :],
                                         in_=denb[0:HD, :, :])
        for sub in range(2):
            nc.vector.tensor_mul(
                ao[sub * HD:(sub + 1) * HD, hp, t0:t0 + N],
                avp[0:HD, sub, :], rvb[0:HD, sub, :])

    def oproj_chain(g, m):
        st = state[g]
        ao = st["ao"]
        psf = ps_proj.tile([P, GT], F32, tag="acc")
        for k2 in range(M):
            nc.tensor.matmul(psf[:], Wp_sb[k2][:, m * P:(m + 1) * P],
                             ao[:, k2, :], start=(k2 == 0), stop=(k2 == M - 1))
        osb = outp.tile([P, GT], BF, tag="osb")
        nc.scalar.add(osb[:], psf[:], add=bprojT[:, m:m + 1])
        nc.sync.dma_start(
            io["outT"][m * P:(m + 1) * P, g * GT:(g + 1) * GT], osb[:])

    def attn_items(g):
        items = []
        for bb in range(NB):
            for hp in range(M):
                items.append(("L", g, bb, hp))
                if hp >= 1:
                    items.append(("A", g, bb, hp - 1))
            items.append(("A", g, bb, M - 1))
        return items

    def oproj_items(g):
        return [("O", g, m) for m in range(M)]

    def proj_items(g):
        items = []
        for m in range(M):
            items.append(("Pq", g, m))
            items.append(("Pk", g, m))
        for i in range(M):
            items.append(("Pv", g, i))
        return items

    def emit(item):
        kind, g, *rest = item
        if kind == "Pq":
            proj_chain(g, "q", rest[0])
        elif kind == "Pk":
            proj_chain(g, "k", rest[0])
        elif kind == "Pv":
            proj_chain(g, "v", rest[0])
        elif kind == "L":
            logits_step(g, rest[0], rest[1])
        elif kind == "A":
            av_step(g, rest[0], rest[1])
        elif kind == "O":
            oproj_chain(g, rest[0])

    # ---- software-pipelined emission: proj(g) | attn(g-1) | oproj(g-2) ----
    for it in range(G + 2):
        g_proj = it if it < G else None
        g_attn = it - 1 if 1 <= it <= G else None
        g_out = it - 2 if it >= 2 else None
        if g_proj is not None:
            stage_group(g_proj + 1)
            make_group(g_proj)
            items_p = proj_items(g_proj)
        else:
            items_p = []
        items_o = oproj_items(g_out) if g_out is not None else []
        items_a = attn_items(g_attn) if g_attn is not None else []
        for item in _merge(items_a, _merge(items_p, items_o)):
            emit(item)
        flush_rot()
        if g_proj is not None:
            staged.pop(g_proj, None)
        if g_out is not None:
            state.pop(g_out, None)


_CACHED_NC = None


def _build_nc():
    global _CACHED_NC
    if _CACHED_NC is not None:
        return _CACHED_NC
    nc = bacc.Bacc("TRN2", target_bir_lowering=False, debug=False,
                   num_devices=NCORES)
    io = {}
    def din(name, shape, dt=BF):
        io[name] = nc.dram_tensor(name, shape, dt, kind="ExternalInput").ap()
    din("qT", [C, T])
    din("kvT", [SEM, T])
    din("Wq", [C, C])
    din("Wk", [SEM, C])
    din("Wv", [SEM, C])
    din("Wproj", [C, C])
    din("cosq", [P, GT]); din("sinq", [P, GT])
    din("cosk", [P, GT]); din("sink", [P, GT])
    din("RT2", [P, P])
    din("bprojT", [P, M], F32)
    io["outT"] = nc.dram_tensor("outT", [C, T], BF, kind="ExternalOutput").ap()

    with tile.TileContext(nc) as tc:
        with ExitStack() as ctx:
            _body(ctx, tc, io)
    nc.compile()
    _CACHED_NC = nc
    return nc


def kernel(q, kv, Wq, Wkv, Wproj, bproj, _trace=False, _trace_kwargs=None):
    nc = _build_nc()
    consts = _host_constants()
    shared = {
        "Wq": np.ascontiguousarray(Wq.astype(bf16)),
        "Wk": np.ascontiguousarray(Wkv[:, :C].astype(bf16)),
        "Wv": np.ascontiguousarray(Wkv[:, C:].astype(bf16)),
        "Wproj": np.ascontiguousarray(Wproj.astype(bf16)),
        "bprojT": np.ascontiguousarray(
            bproj.astype(np.float32).reshape(M, P).T),
        **consts,
    }
    in_maps = []
    for i in range(NCORES):
        qs = q[i * BPC:(i + 1) * BPC].reshape(T, C)
        kvs = kv[i * BPC:(i + 1) * BPC].reshape(T, SEM)
        in_maps.append({
            "qT": np.ascontiguousarray(qs.T.astype(bf16)),
            "kvT": np.ascontiguousarray(kvs.T.astype(bf16)),
            **shared,
        })
    kw = {}
    if _trace:
        kw.update(trace=True, **(_trace_kwargs or {}))
    res = run_bass_kernel_spmd(nc, in_maps, core_ids=list(range(NCORES)), **kw)
    out = np.empty((B, N, C), np.float32)
    for i in range(NCORES):
        out[i * BPC:(i + 1) * BPC] = (
            res.results[i]["outT"].astype(np.float32).T.reshape(BPC, N, C))
    if _trace:
        return out, res
    return out

